# revision 38
# baseline (speedup 1.0000x reference)
"""DPLR-SSM layer kernel for Trainium2 (8 NeuronCores, batch-parallel).

Math: the reference recurrence is
    x_t = M x_{t-1} + B_bar u_t,   M = diag(A_bar) + dt * P Q^H   (n=64 complex)
    y_t = Re(C x_t) + D * u_t
M is time-invariant, so we eigendecompose M = V diag(w) V^{-1} on the host
(tiny, n=64) and run the diagonal system
    x'_t = w x'_{t-1} + B_eff u_t,  y_t = Re(C_eff x'_t) + D u_t
with B_eff = V^{-1} B_bar, C_eff = C V.  The complex diagonal scan is made
real by the phase-rotation trick: with w = rho * e^{i*theta},
z_t = e^{-i*theta*t} x'_t obeys  z_t = rho * z_{t-1} + e^{-i*theta*t} b_t,
which is two independent REAL first-order scans (hardware tensor_tensor_scan,
fp32 carry state, bf16 storage).

Per-core layout (2 batches of the 16), all heavy tensors bf16:
  - u is uploaded twice, already in bf16: pre-transposed [d, t] (PE matmul
    operand for the B-projection, no on-device transposes) and as the
    feedthrough term du = D*u in natural [t, d] layout (D is a constant
    diagonal, folded into input preparation on the host).
  - B-projection: binb[(comp,n), t] = sum_d Bc[d,(comp,n)] * uT[d, t]  (PE,
    4 accumulating chunk matmuls per 512-t group); binb2 is the
    component-swapped projection (second matmul set).
  - pre-rotation (plain 2D per-batch ops so the DVE 2x bf16 mode engages):
        bp = [c;c] * binb - [-s;s] * binb2 = [c*br + s*bi ; c*bi - s*br]
    tmp2 ( [-s;s]*binb2 ) and the post-rotation g2 run on GpSimd, which is
    otherwise idle, to shorten the DVE critical path.
  - hardware scans (state chained across passes via zprev column).
  - post-rotation G1 = [c;s]*z, G2 = [-s;c]*z; C-projection
    y[t,d] = G1.T W1 + G2.T W2 accumulated in PSUM.
  - merge: ACT evacuates the C-proj PSUM to bf16, DVE adds du in an
    all-bf16 2x op, and the store DMA casts bf16->fp32 (software DGE on
    gpsimd; y is written in a pre-tiled layout the host de-tiles).
Processed as Q=4 time-quarter passes with pass q+1's loads issued ahead of
pass q's stores (loads on the sync queue, stores on the gpsimd queue).
"""

import math

import numpy as np

N = 64
D = 512
BATCH = 16
SEQ = 4096
NCORES = 8
BPC = BATCH // NCORES  # batches per core = 2

_PROG_CACHE = {}

# Set by test harnesses to capture a hardware profile; harmless defaults.
TRACE = False
LAST_RESULTS = None


def _host_precompute(log_neg_real, imag, P_real, P_imag, Q_real, Q_imag,
                     B_real, B_imag, C_real, C_imag, log_dt, D_vec, L):
    """All small-parameter math in float64 on host; returns device arrays."""
    import ml_dtypes
    bf16 = ml_dtypes.bfloat16

    dt = math.exp(float(np.asarray(log_dt).reshape(-1)[0]))
    Lam = -np.exp(log_neg_real.astype(np.float64)) + 1j * imag.astype(np.float64)
    A_bar = np.exp(Lam * dt)
    B = B_real.astype(np.float64) + 1j * B_imag.astype(np.float64)
    B_bar = ((A_bar - 1.0) / (Lam + 1e-8) * dt)[:, None] * B          # (n, d)
    P = P_real.astype(np.float64) + 1j * P_imag.astype(np.float64)
    Qc = Q_real.astype(np.float64) - 1j * Q_imag.astype(np.float64)
    C = C_real.astype(np.float64) + 1j * C_imag.astype(np.float64)   # (d, n)

    M = np.diag(A_bar) + dt * (P @ Qc.T)
    w, V = np.linalg.eig(M)
    B_eff = np.linalg.solve(V, B_bar)                                 # (n, d)
    C_eff = C @ V                                                     # (d, n)

    rho = np.abs(w)
    theta = np.angle(w)
    t_idx = np.arange(1, L + 1, dtype=np.float64)
    ang = np.outer(theta, t_idx)                                      # (n, L)
    cos_t = np.cos(ang)
    sin_t = np.sin(ang)

    # post-rotation tables (128, L): t1 = [cos; sin], t2 = [-sin; cos]
    t1 = np.concatenate([cos_t, sin_t], axis=0).astype(bf16)
    t2 = np.concatenate([-sin_t, cos_t], axis=0).astype(bf16)
    # pre-rotation table for tmp2: tsn = [-sin; sin]  (tmp1's [cos; cos]
    # comes from half-width ops on t1/t2 instead of a fourth table)
    tsn = np.concatenate([-sin_t, sin_t], axis=0).astype(bf16)

    # rho column (128, 1) fp32: per-partition scan coefficient
    rhoc = np.concatenate([rho, rho]).astype(np.float32).reshape(128, 1)

    # B weights, lhsT layout: bcomb[p, c*128+m] = Bc[c*128+p, m]
    # where Bc[d, m] with m=comp*64+n: comp0 -> Re(B_eff)[n,d], comp1 -> Im
    Bc = np.concatenate([B_eff.real, B_eff.imag], axis=0).T           # (512, 128)
    bcomb = Bc.reshape(4, 128, 128).transpose(1, 0, 2).reshape(128, 512)
    bcomb = np.ascontiguousarray(bcomb).astype(bf16)
    # component-swapped variant [bi ; br]
    Bc2 = np.concatenate([B_eff.imag, B_eff.real], axis=0).T          # (512, 128)
    bcomb2 = Bc2.reshape(4, 128, 128).transpose(1, 0, 2).reshape(128, 512)
    bcomb2 = np.ascontiguousarray(bcomb2).astype(bf16)

    # C-proj weights: W1 = [Cr; -Cr], W2 = [Ci; -Ci]
    Cr = C_eff.real.T                                                 # (n, d)
    Ci = C_eff.imag.T
    W1 = np.concatenate([Cr, -Cr], axis=0)                            # (128, 512)
    W2 = np.concatenate([Ci, -Ci], axis=0)
    cexp = np.concatenate([W1, W2], axis=1).astype(bf16)              # (128, 1024)

    return dict(t1=t1, t2=t2, tsn=tsn, rhoc=rhoc,
                bcomb=bcomb, bcomb2=bcomb2, cexp=cexp)


def _split_multi_waits(nc, mybir):
    """Walrus codegen only honors a single sync-wait slot on compute
    instruction structs (ACT/TS/TT...).  Move surplus waits onto chained
    EventSemaphore instructions on the same engine right before the op --
    in-order engine execution makes this equivalent."""
    n = 0
    for func in nc.m.functions:
        for blk in func.blocks:
            il = blk.instructions
            i = 0
            while i < len(il):
                inst = il[i]
                si = inst.sync_info
                if (si is not None and si.on_wait and len(si.on_wait) > 1
                        and not isinstance(inst, mybir.InstEventSemaphore)):
                    waits = list(si.on_wait)
                    for w in waits[:-1]:
                        ev = mybir.InstEventSemaphore(
                            name=f"EVW-{n}", ins=[], outs=[])
                        n += 1
                        ev.engine = inst.engine
                        ev.sync_info = mybir.SyncInfo(on_wait=[w],
                                                      on_update=[])
                        il.insert(i, ev)
                        i += 1
                    inst.sync_info = mybir.SyncInfo(on_wait=[waits[-1]],
                                                    on_update=si.on_update)
                i += 1
    return n


def _build_program(L, split_waits=True):
    """SPMD Bass program for one core: u (BPC*L, 512) -> y (BPC*L, 512) fp32.
    Q=4 time-quarter passes, loads for pass q+1 prefetched before pass q's
    compute is emitted."""
    import concourse.bass as bass
    import concourse.mybir as mybir
    import concourse.tile as tile

    TROWS = BPC * L            # 8192 time-rows per core
    Q = 4                      # passes (time quarters)
    TQ = L // Q                # 1024 time steps per pass per batch
    FP32 = mybir.dt.float32
    BF16 = mybir.dt.bfloat16
    Alu = mybir.AluOpType

    nc = bass.Bass()
    # du/ut/y are tiled on the host so each per-pass DMA is one plain 2D
    # transfer with a KB-contiguous row per partition (128 descriptors):
    #   du row (q*BPC+b)*128+p holds du[b, q*TQ + j*128 + p, :] for j=0..7
    #   ut row (q*BPC+b)*128+p holds u[b, q*TQ + t, c*128+p] g-major:
    #     col g*2048 + c*512 + t' = u[.., q*TQ + g*512 + t', c*128+p]
    #   y  row (q*BPC+b)*128+p holds y[b, q*TQ + j*128 + p, :] for j=0..7
    du_d = nc.dram_tensor("du", [Q * BPC * 128, 8 * 512], BF16,
                          kind="ExternalInput")
    ut_d = nc.dram_tensor("ut", [Q * BPC * 128, 4 * (L // Q)], BF16,
                          kind="ExternalInput")
    t1_d = nc.dram_tensor("t1", [128, L], BF16, kind="ExternalInput")
    t2_d = nc.dram_tensor("t2", [128, L], BF16, kind="ExternalInput")
    tsn_d = nc.dram_tensor("tsn", [128, L], BF16, kind="ExternalInput")
    rhoc_d = nc.dram_tensor("rhoc", [128, 1], FP32, kind="ExternalInput")
    bcomb_d = nc.dram_tensor("bcomb", [128, 512], BF16, kind="ExternalInput")
    bcomb2_d = nc.dram_tensor("bcomb2", [128, 512], BF16, kind="ExternalInput")
    cexp_d = nc.dram_tensor("cexp", [128, 1024], BF16, kind="ExternalInput")
    y_d = nc.dram_tensor("y", [Q * BPC * 128, 8 * 512], FP32,
                         kind="ExternalOutput")

    with tile.TileContext(nc) as tc_:
        with (
            tc_.tile_pool(name="persist", bufs=1) as pp,
            tc_.tile_pool(name="ptab", bufs=2) as ptab,
            tc_.tile_pool(name="put", bufs=4) as put,
            tc_.tile_pool(name="pun", bufs=4) as pun,
            tc_.tile_pool(name="pbin", bufs=2) as pbin,
            tc_.tile_pool(name="ptmp", bufs=2) as ptmp,
            tc_.tile_pool(name="pz", bufs=2) as pz,
            tc_.tile_pool(name="pg", bufs=2) as pg,
            tc_.tile_pool(name="pyo", bufs=7) as pyo,
            tc_.tile_pool(name="pys", bufs=4) as pys,
            tc_.tile_pool(name="psB", bufs=1, space="PSUM") as psB,
            tc_.tile_pool(name="psC", bufs=3, space="PSUM") as psC,
        ):
            bcomb_s = pp.tile([128, 512], BF16, tag="bcomb")
            bcomb2_s = pp.tile([128, 512], BF16, tag="bcomb2")
            cexp_s = pp.tile([128, 1024], BF16, tag="cexp")
            rhoc_s = pp.tile([128, 1], FP32, tag="rhoc")
            zprev = pp.tile([128, BPC], FP32, tag="zprev")
            # small persistent params go on the scalar queue so the sync
            # queue starts streaming pass-0 u immediately
            nc.scalar.dma_start(out=bcomb_s, in_=bcomb_d[:, :])
            nc.scalar.dma_start(out=bcomb2_s, in_=bcomb2_d[:, :])
            nc.scalar.dma_start(out=cexp_s, in_=cexp_d[:, :])
            nc.scalar.dma_start(out=rhoc_s, in_=rhoc_d[:, :])
            nc.gpsimd.memset(zprev, 0.0)

            rhob = rhoc_s.broadcast_to([128, TQ])

            def emit_loads(q):
                """u loads (sync queue) + table loads (scalar queue)."""
                cs = slice(q * TQ, (q + 1) * TQ)
                tiles = {}
                for b in range(BPC):
                    uT = put.tile([128, 4 * TQ], BF16, tag="uT")
                    rows = slice((q * BPC + b) * 128, (q * BPC + b + 1) * 128)
                    # two halves so the first B-proj group can start after
                    # half the tile has landed (g-major layout)
                    half = 2 * TQ
                    nc.sync.dma_start(out=uT[:, 0:half],
                                      in_=ut_d[rows, 0:half])
                    nc.sync.dma_start(out=uT[:, half:2 * half],
                                      in_=ut_d[rows, half:2 * half])
                    tiles[("uT", b)] = uT
                for b in range(BPC):
                    du = pun.tile([128, 8 * 512], BF16, tag="du")
                    rows = slice((q * BPC + b) * 128, (q * BPC + b + 1) * 128)
                    nc.sync.dma_start(out=du, in_=du_d[rows, :])
                    tiles[("du", b)] = du
                for name, dram in (("t1", t1_d), ("t2", t2_d),
                                   ("tsn", tsn_d)):
                    ts_ = ptab.tile([128, TQ], BF16, tag=name)
                    nc.sync.dma_start(out=ts_, in_=dram[:, cs])
                    tiles[name] = ts_
                return tiles

            tiles = emit_loads(0)
            for q in range(Q):
                nxt = emit_loads(q + 1) if q + 1 < Q else None
                cur = tiles

                # ---------------- B-projection (PE) ------------------------
                binb_s = pbin.tile([128, BPC * TQ], BF16, tag="binb")
                binb2_s = pbin.tile([128, BPC * TQ], BF16, tag="binb2")
                for b in range(BPC):
                    uT = cur[("uT", b)]
                    for g in range(TQ // 512):
                        pb = psB.tile([128, 512], FP32, tag="pb")
                        pb2 = psB.tile([128, 512], FP32, tag="pb2")
                        for c in range(4):
                            rhs = uT[:, g * 2048 + c * 512:
                                     g * 2048 + (c + 1) * 512]
                            nc.tensor.matmul(
                                pb, bcomb_s[:, c * 128:(c + 1) * 128], rhs,
                                start=(c == 0), stop=(c == 3))
                        for c in range(4):
                            rhs = uT[:, g * 2048 + c * 512:
                                     g * 2048 + (c + 1) * 512]
                            nc.tensor.matmul(
                                pb2, bcomb2_s[:, c * 128:(c + 1) * 128], rhs,
                                start=(c == 0), stop=(c == 3))
                        off = b * TQ + g * 512
                        nc.scalar.copy(binb_s[:, off:off + 512], pb)
                        nc.scalar.copy(binb2_s[:, off:off + 512], pb2)

                # ---------------- rotation + scan (DVE + GpSimd) -----------
                # plain 2D per-batch slices keep the DVE 2x bf16 mode; the
                # tmp2/g2 products run on the otherwise-idle GpSimd
                tmp1 = ptmp.tile([128, BPC * TQ], BF16, tag="tmp1")
                tmp2 = ptmp.tile([128, BPC * TQ], BF16, tag="tmp2")
                sls = [slice(b * TQ, (b + 1) * TQ) for b in range(BPC)]
                for b in range(BPC):
                    # tmp1 = [cos;cos]*binb via aligned half-lane products:
                    # t1 rows 0:64 and t2 rows 64:128 are both cos
                    nc.vector.tensor_mul(tmp1[0:64, sls[b]],
                                         cur["t1"][0:64, :],
                                         binb_s[0:64, sls[b]])
                    nc.vector.tensor_mul(tmp1[64:128, sls[b]],
                                         cur["t2"][64:128, :],
                                         binb_s[64:128, sls[b]])
                    nc.gpsimd.tensor_mul(tmp2[:, sls[b]], cur["tsn"][:, :],
                                         binb2_s[:, sls[b]])
                # bp overwrites binb_s (its inputs are tmp1/tmp2); per-batch
                # so scan(b0) is not gated on batch 1's products
                for b in range(BPC):
                    nc.vector.tensor_sub(binb_s[:, sls[b]],
                                         tmp1[:, sls[b]], tmp2[:, sls[b]])

                z = pz.tile([128, BPC * TQ], BF16, tag="z")
                for b in range(BPC):
                    nc.vector.tensor_tensor_scan(
                        z[:, sls[b]], rhob, binb_s[:, sls[b]],
                        zprev[:, b:b + 1], Alu.mult, Alu.add)
                    nc.vector.tensor_copy(
                        zprev[:, b:b + 1],
                        z[:, b * TQ + TQ - 1:b * TQ + TQ])

                g1 = pg.tile([128, BPC * TQ], BF16, tag="g1")
                g2 = pg.tile([128, BPC * TQ], BF16, tag="g2")
                for b in range(BPC):
                    nc.vector.tensor_mul(g1[:, sls[b]], cur["t1"][:, :],
                                         z[:, sls[b]])
                    nc.gpsimd.tensor_mul(g2[:, sls[b]], cur["t2"][:, :],
                                         z[:, sls[b]])

                # ---------------- C-projection + feedthrough + store -------
                # PSUM is evacuated to bf16 (ACT), the du add runs as an
                # all-bf16 DVE 2x op, and the store casts bf16->fp32 in the
                # DMA (software DGE on gpsimd; y is de-tiled on the host)
                for b in range(BPC):
                    du = cur[("du", b)]
                    yrows = slice((q * BPC + b) * 128,
                                  (q * BPC + b + 1) * 128)
                    for h in range(2):        # 512-row output halves
                        yout = pyo.tile([128, 2048], BF16, tag="yout")
                        ysum = pys.tile([128, 2048], BF16, tag="ysum")
                        for jp in range(2):   # pairs of 128-t psum tiles
                            py = psC.tile([128, 1024], FP32, tag="py")
                            for ji in range(2):
                                j = h * 4 + jp * 2 + ji
                                off = b * TQ + j * 128
                                ps_ = py[:, ji * 512:(ji + 1) * 512]
                                nc.tensor.matmul(ps_, g1[:, off:off + 128],
                                                 cexp_s[:, 0:512],
                                                 start=True, stop=False)
                                nc.tensor.matmul(ps_, g2[:, off:off + 128],
                                                 cexp_s[:, 512:1024],
                                                 start=False, stop=True)
                            nc.scalar.copy(
                                ysum[:, jp * 1024:(jp + 1) * 1024], py)
                        # one merged all-bf16 2x add; the store waits for
                        # both jp halves either way
                        nc.vector.tensor_add(
                            yout, ysum, du[:, h * 2048:(h + 1) * 2048])
                        nc.gpsimd.dma_start(
                            out=y_d[yrows, h * 2048:(h + 1) * 2048],
                            in_=yout)

                tiles = nxt

    if split_waits:
        _split_multi_waits(nc, mybir)
    return nc


def kernel(**inputs):
    import ml_dtypes
    from concourse.bass_utils import run_bass_kernel_spmd

    bf16 = ml_dtypes.bfloat16
    u = np.ascontiguousarray(inputs["u"], dtype=np.float32)
    L = u.shape[1]
    params = _host_precompute(
        inputs["log_neg_real"], inputs["imag"], inputs["P_real"],
        inputs["P_imag"], inputs["Q_real"], inputs["Q_imag"],
        inputs["B_real"], inputs["B_imag"], inputs["C_real"],
        inputs["C_imag"], inputs["log_dt"], inputs["D"], L)

    if L not in _PROG_CACHE:
        _PROG_CACHE[L] = _build_program(L)
    nc = _PROG_CACHE[L]

    u16 = u.astype(bf16)
    du16 = (u * inputs["D"].astype(np.float32)[None, None, :]).astype(bf16)
    Q = 4
    TQ = L // Q
    in_maps = []
    for c in range(NCORES):
        shard = u16[c * BPC:(c + 1) * BPC].reshape(BPC * L, u.shape[2])
        dshard = du16[c * BPC:(c + 1) * BPC].reshape(BPC * L, u.shape[2])
        # pre-tiled layouts (see _build_program): one 8KB-contiguous row per
        # (pass, batch, partition)
        dut = dshard.reshape(BPC, Q, 8, 128, 512).transpose(
            1, 0, 3, 2, 4).reshape(Q * BPC * 128, 8 * 512)
        utt = np.ascontiguousarray(shard.T).reshape(
            4, 128, BPC, Q, 2, 512).transpose(
            3, 2, 1, 4, 0, 5).reshape(Q * BPC * 128, 4 * TQ)
        m = {"du": np.ascontiguousarray(dut),
             "ut": np.ascontiguousarray(utt)}
        m.update(params)
        in_maps.append(m)

    kwargs = {}
    if TRACE:
        kwargs = dict(trace=True, stitch_traces=False)
    res = run_bass_kernel_spmd(nc, in_maps, core_ids=list(range(NCORES)),
                               **kwargs)
    global LAST_RESULTS
    LAST_RESULTS = res
    y = np.empty_like(u)
    for c in range(NCORES):
        yt = res.results[c]["y"].reshape(Q, BPC, 128, 8, 512)
        y[c * BPC:(c + 1) * BPC] = yt.transpose(1, 0, 3, 2, 4).reshape(
            BPC, L, u.shape[2])
    return y


# revision 39
# speedup vs baseline: 1.0674x; 1.0674x over previous
"""DPLR-SSM layer kernel for Trainium2 (8 NeuronCores, batch-parallel).

Math: the reference recurrence is
    x_t = M x_{t-1} + B_bar u_t,   M = diag(A_bar) + dt * P Q^H   (n=64 complex)
    y_t = Re(C x_t) + D * u_t
M is time-invariant, so we eigendecompose M = V diag(w) V^{-1} on the host
(tiny, n=64) and run the diagonal system
    x'_t = w x'_{t-1} + B_eff u_t,  y_t = Re(C_eff x'_t) + D u_t
with B_eff = V^{-1} B_bar, C_eff = C V.  The complex diagonal scan is made
real by the phase-rotation trick: with w = rho * e^{i*theta},
z_t = e^{-i*theta*t} x'_t obeys  z_t = rho * z_{t-1} + e^{-i*theta*t} b_t,
which is two independent REAL first-order scans (hardware tensor_tensor_scan,
fp32 carry state, bf16 storage).

Per-core layout (2 batches of the 16), all heavy tensors bf16:
  - u is uploaded twice, already in bf16: pre-transposed [d, t] (PE matmul
    operand for the B-projection, no on-device transposes) and as the
    feedthrough term du = D*u in natural [t, d] layout (D is a constant
    diagonal, folded into input preparation on the host).
  - B-projection: binb[(comp,n), t] = sum_d Bc[d,(comp,n)] * uT[d, t]  (PE,
    4 accumulating chunk matmuls per 512-t group); binb2 is the
    component-swapped projection (second matmul set).
  - pre-rotation (plain 2D per-batch ops so the DVE 2x bf16 mode engages):
        bp = [c;c] * binb - [-s;s] * binb2 = [c*br + s*bi ; c*bi - s*br]
    tmp2 ( [-s;s]*binb2 ) and the post-rotation g2 run on GpSimd, which is
    otherwise idle, to shorten the DVE critical path.
  - hardware scans (state chained across passes via zprev column).
  - post-rotation G1 = [c;s]*z, G2 = [-s;c]*z; C-projection
    y[t,d] = G1.T W1 + G2.T W2 accumulated in PSUM.
  - merge: ACT evacuates the C-proj PSUM to bf16, DVE adds du in an
    all-bf16 2x op, and the store DMA casts bf16->fp32 (software DGE on
    gpsimd; y is written in a pre-tiled layout the host de-tiles).
Processed as Q=4 time-quarter passes with pass q+1's loads issued ahead of
pass q's stores (loads on the sync queue, stores on the gpsimd queue).
"""

import math

import numpy as np

N = 64
D = 512
BATCH = 16
SEQ = 4096
NCORES = 8
BPC = BATCH // NCORES  # batches per core = 2

_PROG_CACHE = {}

# Set by test harnesses to capture a hardware profile; harmless defaults.
TRACE = False
LAST_RESULTS = None


def _host_precompute(log_neg_real, imag, P_real, P_imag, Q_real, Q_imag,
                     B_real, B_imag, C_real, C_imag, log_dt, D_vec, L):
    """All small-parameter math in float64 on host; returns device arrays."""
    import ml_dtypes
    bf16 = ml_dtypes.bfloat16

    dt = math.exp(float(np.asarray(log_dt).reshape(-1)[0]))
    Lam = -np.exp(log_neg_real.astype(np.float64)) + 1j * imag.astype(np.float64)
    A_bar = np.exp(Lam * dt)
    B = B_real.astype(np.float64) + 1j * B_imag.astype(np.float64)
    B_bar = ((A_bar - 1.0) / (Lam + 1e-8) * dt)[:, None] * B          # (n, d)
    P = P_real.astype(np.float64) + 1j * P_imag.astype(np.float64)
    Qc = Q_real.astype(np.float64) - 1j * Q_imag.astype(np.float64)
    C = C_real.astype(np.float64) + 1j * C_imag.astype(np.float64)   # (d, n)

    M = np.diag(A_bar) + dt * (P @ Qc.T)
    w, V = np.linalg.eig(M)
    B_eff = np.linalg.solve(V, B_bar)                                 # (n, d)
    C_eff = C @ V                                                     # (d, n)

    rho = np.abs(w)
    theta = np.angle(w)
    t_idx = np.arange(1, L + 1, dtype=np.float64)
    ang = np.outer(theta, t_idx)                                      # (n, L)
    cos_t = np.cos(ang)
    sin_t = np.sin(ang)

    # post-rotation tables (128, L): t1 = [cos; sin], t2 = [-sin; cos]
    t1 = np.concatenate([cos_t, sin_t], axis=0).astype(bf16)
    t2 = np.concatenate([-sin_t, cos_t], axis=0).astype(bf16)
    # pre-rotation table for tmp2: tsn = [-sin; sin]  (tmp1's [cos; cos]
    # comes from half-width ops on t1/t2 instead of a fourth table)
    tsn = np.concatenate([-sin_t, sin_t], axis=0).astype(bf16)

    # rho column (128, 1) fp32: per-partition scan coefficient
    rhoc = np.concatenate([rho, rho]).astype(np.float32).reshape(128, 1)

    # B weights, lhsT layout: bcomb[p, c*128+m] = Bc[c*128+p, m]
    # where Bc[d, m] with m=comp*64+n: comp0 -> Re(B_eff)[n,d], comp1 -> Im
    Bc = np.concatenate([B_eff.real, B_eff.imag], axis=0).T           # (512, 128)
    bcomb = Bc.reshape(4, 128, 128).transpose(1, 0, 2).reshape(128, 512)
    bcomb = np.ascontiguousarray(bcomb).astype(bf16)
    # component-swapped variant [bi ; br]
    Bc2 = np.concatenate([B_eff.imag, B_eff.real], axis=0).T          # (512, 128)
    bcomb2 = Bc2.reshape(4, 128, 128).transpose(1, 0, 2).reshape(128, 512)
    bcomb2 = np.ascontiguousarray(bcomb2).astype(bf16)

    # C-proj weights: W1 = [Cr; -Cr], W2 = [Ci; -Ci]
    Cr = C_eff.real.T                                                 # (n, d)
    Ci = C_eff.imag.T
    W1 = np.concatenate([Cr, -Cr], axis=0)                            # (128, 512)
    W2 = np.concatenate([Ci, -Ci], axis=0)
    cexp = np.concatenate([W1, W2], axis=1).astype(bf16)              # (128, 1024)

    return dict(t1=t1, t2=t2, tsn=tsn, rhoc=rhoc,
                bcomb=bcomb, bcomb2=bcomb2, cexp=cexp)


def _split_multi_waits(nc, mybir):
    """Walrus codegen only honors a single sync-wait slot on compute
    instruction structs (ACT/TS/TT...).  Move surplus waits onto chained
    EventSemaphore instructions on the same engine right before the op --
    in-order engine execution makes this equivalent."""
    n = 0
    for func in nc.m.functions:
        for blk in func.blocks:
            il = blk.instructions
            i = 0
            while i < len(il):
                inst = il[i]
                si = inst.sync_info
                if (si is not None and si.on_wait and len(si.on_wait) > 1
                        and not isinstance(inst, mybir.InstEventSemaphore)):
                    waits = list(si.on_wait)
                    for w in waits[:-1]:
                        ev = mybir.InstEventSemaphore(
                            name=f"EVW-{n}", ins=[], outs=[])
                        n += 1
                        ev.engine = inst.engine
                        ev.sync_info = mybir.SyncInfo(on_wait=[w],
                                                      on_update=[])
                        il.insert(i, ev)
                        i += 1
                    inst.sync_info = mybir.SyncInfo(on_wait=[waits[-1]],
                                                    on_update=si.on_update)
                i += 1
    return n


def _build_program(L, split_waits=True):
    """SPMD Bass program for one core: u (BPC*L, 512) -> y (BPC*L, 512) fp32.
    Q=4 time-quarter passes, loads for pass q+1 prefetched before pass q's
    compute is emitted."""
    import concourse.bass as bass
    import concourse.mybir as mybir
    import concourse.tile as tile

    TROWS = BPC * L            # 8192 time-rows per core
    Q = 4                      # passes (time quarters)
    TQ = L // Q                # 1024 time steps per pass per batch
    FP32 = mybir.dt.float32
    BF16 = mybir.dt.bfloat16
    Alu = mybir.AluOpType

    nc = bass.Bass()
    # du/ut/y are tiled on the host so each per-pass DMA is one plain 2D
    # transfer with a KB-contiguous row per partition (128 descriptors):
    #   du row (q*BPC+b)*128+p holds du[b, q*TQ + j*128 + p, :] for j=0..7
    #   ut row (q*BPC+b)*128+p holds u[b, q*TQ + t, c*128+p] g-major:
    #     col g*2048 + c*512 + t' = u[.., q*TQ + g*512 + t', c*128+p]
    #   y  row (q*BPC+b)*128+p holds y[b, q*TQ + j*128 + p, :] for j=0..7
    du_d = nc.dram_tensor("du", [Q * BPC * 128, 8 * 512], BF16,
                          kind="ExternalInput")
    ut_d = nc.dram_tensor("ut", [Q * BPC * 128, 4 * (L // Q)], BF16,
                          kind="ExternalInput")
    t1_d = nc.dram_tensor("t1", [128, L], BF16, kind="ExternalInput")
    t2_d = nc.dram_tensor("t2", [128, L], BF16, kind="ExternalInput")
    tsn_d = nc.dram_tensor("tsn", [128, L], BF16, kind="ExternalInput")
    rhoc_d = nc.dram_tensor("rhoc", [128, 1], FP32, kind="ExternalInput")
    bcomb_d = nc.dram_tensor("bcomb", [128, 512], BF16, kind="ExternalInput")
    bcomb2_d = nc.dram_tensor("bcomb2", [128, 512], BF16, kind="ExternalInput")
    cexp_d = nc.dram_tensor("cexp", [128, 1024], BF16, kind="ExternalInput")
    y_d = nc.dram_tensor("y", [Q * BPC * 128, 8 * 512], FP32,
                         kind="ExternalOutput")

    with tile.TileContext(nc) as tc_:
        with (
            tc_.tile_pool(name="persist", bufs=1) as pp,
            tc_.tile_pool(name="ptab", bufs=2) as ptab,
            tc_.tile_pool(name="put", bufs=4) as put,
            tc_.tile_pool(name="pun", bufs=4) as pun,
            tc_.tile_pool(name="pbin", bufs=2) as pbin,
            tc_.tile_pool(name="ptmp", bufs=2) as ptmp,
            tc_.tile_pool(name="pz", bufs=2) as pz,
            tc_.tile_pool(name="pg", bufs=2) as pg,
            tc_.tile_pool(name="pyo", bufs=7) as pyo,
            tc_.tile_pool(name="pys", bufs=4) as pys,
            tc_.tile_pool(name="psB", bufs=1, space="PSUM") as psB,
            tc_.tile_pool(name="psC", bufs=3, space="PSUM") as psC,
        ):
            bcomb_s = pp.tile([128, 512], BF16, tag="bcomb")
            bcomb2_s = pp.tile([128, 512], BF16, tag="bcomb2")
            cexp_s = pp.tile([128, 1024], BF16, tag="cexp")
            rhoc_s = pp.tile([128, 1], FP32, tag="rhoc")
            zprev = pp.tile([128, BPC], FP32, tag="zprev")
            # small persistent params go on the scalar queue so the sync
            # queue starts streaming pass-0 u immediately
            nc.scalar.dma_start(out=bcomb_s, in_=bcomb_d[:, :])
            nc.scalar.dma_start(out=bcomb2_s, in_=bcomb2_d[:, :])
            nc.scalar.dma_start(out=cexp_s, in_=cexp_d[:, :])
            nc.scalar.dma_start(out=rhoc_s, in_=rhoc_d[:, :])
            nc.gpsimd.memset(zprev, 0.0)

            rhob = rhoc_s.broadcast_to([128, TQ])

            def emit_loads(q):
                """u loads (sync queue) + table loads (scalar queue)."""
                cs = slice(q * TQ, (q + 1) * TQ)
                tiles = {}
                for b in range(BPC):
                    uT = put.tile([128, 4 * TQ], BF16, tag="uT")
                    rows = slice((q * BPC + b) * 128, (q * BPC + b + 1) * 128)
                    # two halves so the first B-proj group can start after
                    # half the tile has landed (g-major layout)
                    half = 2 * TQ
                    nc.sync.dma_start(out=uT[:, 0:half],
                                      in_=ut_d[rows, 0:half])
                    nc.sync.dma_start(out=uT[:, half:2 * half],
                                      in_=ut_d[rows, half:2 * half])
                    tiles[("uT", b)] = uT
                for b in range(BPC):
                    du = pun.tile([128, 8 * 512], BF16, tag="du")
                    rows = slice((q * BPC + b) * 128, (q * BPC + b + 1) * 128)
                    nc.sync.dma_start(out=du, in_=du_d[rows, :])
                    tiles[("du", b)] = du
                for name, dram in (("t1", t1_d), ("t2", t2_d),
                                   ("tsn", tsn_d)):
                    ts_ = ptab.tile([128, TQ], BF16, tag=name)
                    nc.sync.dma_start(out=ts_, in_=dram[:, cs])
                    tiles[name] = ts_
                return tiles

            tiles = emit_loads(0)
            for q in range(Q):
                nxt = emit_loads(q + 1) if q + 1 < Q else None
                cur = tiles

                # ---------------- B-projection (PE) ------------------------
                binb_s = pbin.tile([128, BPC * TQ], BF16, tag="binb")
                binb2_s = pbin.tile([128, BPC * TQ], BF16, tag="binb2")
                for b in range(BPC):
                    uT = cur[("uT", b)]
                    for g in range(TQ // 512):
                        pb = psB.tile([128, 512], FP32, tag="pb")
                        pb2 = psB.tile([128, 512], FP32, tag="pb2")
                        for c in range(4):
                            rhs = uT[:, g * 2048 + c * 512:
                                     g * 2048 + (c + 1) * 512]
                            nc.tensor.matmul(
                                pb, bcomb_s[:, c * 128:(c + 1) * 128], rhs,
                                start=(c == 0), stop=(c == 3))
                        for c in range(4):
                            rhs = uT[:, g * 2048 + c * 512:
                                     g * 2048 + (c + 1) * 512]
                            nc.tensor.matmul(
                                pb2, bcomb2_s[:, c * 128:(c + 1) * 128], rhs,
                                start=(c == 0), stop=(c == 3))
                        off = b * TQ + g * 512
                        nc.scalar.copy(binb_s[:, off:off + 512], pb)
                        nc.scalar.copy(binb2_s[:, off:off + 512], pb2)

                # ---------------- rotation + scan (DVE + GpSimd) -----------
                # plain 2D per-batch slices keep the DVE 2x bf16 mode; the
                # tmp2/g2 products run on the otherwise-idle GpSimd
                tmp1 = ptmp.tile([128, BPC * TQ], BF16, tag="tmp1")
                tmp2 = ptmp.tile([128, BPC * TQ], BF16, tag="tmp2")
                sls = [slice(b * TQ, (b + 1) * TQ) for b in range(BPC)]
                for b in range(BPC):
                    # tmp1 = [cos;cos]*binb via aligned half-lane products:
                    # t1 rows 0:64 and t2 rows 64:128 are both cos
                    nc.vector.tensor_mul(tmp1[0:64, sls[b]],
                                         cur["t1"][0:64, :],
                                         binb_s[0:64, sls[b]])
                    nc.vector.tensor_mul(tmp1[64:128, sls[b]],
                                         cur["t2"][64:128, :],
                                         binb_s[64:128, sls[b]])
                    nc.gpsimd.tensor_mul(tmp2[:, sls[b]], cur["tsn"][:, :],
                                         binb2_s[:, sls[b]])
                # bp overwrites binb_s (its inputs are tmp1/tmp2); per-batch
                # so scan(b0) is not gated on batch 1's products
                for b in range(BPC):
                    nc.vector.tensor_sub(binb_s[:, sls[b]],
                                         tmp1[:, sls[b]], tmp2[:, sls[b]])

                z = pz.tile([128, BPC * TQ], BF16, tag="z")
                for b in range(BPC):
                    nc.vector.tensor_tensor_scan(
                        z[:, sls[b]], rhob, binb_s[:, sls[b]],
                        zprev[:, b:b + 1], Alu.mult, Alu.add)
                    nc.vector.tensor_copy(
                        zprev[:, b:b + 1],
                        z[:, b * TQ + TQ - 1:b * TQ + TQ])

                g1 = pg.tile([128, BPC * TQ], BF16, tag="g1")
                g2 = pg.tile([128, BPC * TQ], BF16, tag="g2")
                for b in range(BPC):
                    nc.vector.tensor_mul(g1[:, sls[b]], cur["t1"][:, :],
                                         z[:, sls[b]])
                    nc.gpsimd.tensor_mul(g2[:, sls[b]], cur["t2"][:, :],
                                         z[:, sls[b]])

                # ---------------- C-projection + feedthrough + store -------
                # PSUM is evacuated to bf16 (ACT), the du add runs as an
                # all-bf16 DVE 2x op, and the store casts bf16->fp32 in the
                # DMA (software DGE on gpsimd; y is de-tiled on the host)
                for b in range(BPC):
                    du = cur[("du", b)]
                    yrows = slice((q * BPC + b) * 128,
                                  (q * BPC + b + 1) * 128)
                    for h in range(2):        # 512-row output halves
                        yout = pyo.tile([128, 2048], BF16, tag="yout")
                        for jp in range(2):   # pairs of 128-t psum tiles
                            py = psC.tile([128, 1024], FP32, tag="py")
                            for ji in range(2):
                                j = h * 4 + jp * 2 + ji
                                off = b * TQ + j * 128
                                ps_ = py[:, ji * 512:(ji + 1) * 512]
                                nc.tensor.matmul(ps_, g1[:, off:off + 128],
                                                 cexp_s[:, 0:512],
                                                 start=True, stop=False)
                                nc.tensor.matmul(ps_, g2[:, off:off + 128],
                                                 cexp_s[:, 512:1024],
                                                 start=False, stop=True)
                            # per-jp evac + add: the add for jp0 overlaps
                            # the ACT evac for jp1 (finer pipelining beats
                            # the saved op overhead of a merged add)
                            ysum = pys.tile([128, 1024], BF16, tag="ysum")
                            nc.scalar.copy(ysum, py)
                            js = slice(jp * 1024, (jp + 1) * 1024)
                            nc.vector.tensor_add(
                                yout[:, js], ysum,
                                du[:, h * 2048 + jp * 1024:
                                   h * 2048 + (jp + 1) * 1024])
                        nc.gpsimd.dma_start(
                            out=y_d[yrows, h * 2048:(h + 1) * 2048],
                            in_=yout)

                tiles = nxt

    if split_waits:
        _split_multi_waits(nc, mybir)
    return nc


def kernel(**inputs):
    import ml_dtypes
    from concourse.bass_utils import run_bass_kernel_spmd

    bf16 = ml_dtypes.bfloat16
    u = np.ascontiguousarray(inputs["u"], dtype=np.float32)
    L = u.shape[1]
    params = _host_precompute(
        inputs["log_neg_real"], inputs["imag"], inputs["P_real"],
        inputs["P_imag"], inputs["Q_real"], inputs["Q_imag"],
        inputs["B_real"], inputs["B_imag"], inputs["C_real"],
        inputs["C_imag"], inputs["log_dt"], inputs["D"], L)

    if L not in _PROG_CACHE:
        _PROG_CACHE[L] = _build_program(L)
    nc = _PROG_CACHE[L]

    u16 = u.astype(bf16)
    du16 = (u * inputs["D"].astype(np.float32)[None, None, :]).astype(bf16)
    Q = 4
    TQ = L // Q
    in_maps = []
    for c in range(NCORES):
        shard = u16[c * BPC:(c + 1) * BPC].reshape(BPC * L, u.shape[2])
        dshard = du16[c * BPC:(c + 1) * BPC].reshape(BPC * L, u.shape[2])
        # pre-tiled layouts (see _build_program): one 8KB-contiguous row per
        # (pass, batch, partition)
        dut = dshard.reshape(BPC, Q, 8, 128, 512).transpose(
            1, 0, 3, 2, 4).reshape(Q * BPC * 128, 8 * 512)
        utt = np.ascontiguousarray(shard.T).reshape(
            4, 128, BPC, Q, 2, 512).transpose(
            3, 2, 1, 4, 0, 5).reshape(Q * BPC * 128, 4 * TQ)
        m = {"du": np.ascontiguousarray(dut),
             "ut": np.ascontiguousarray(utt)}
        m.update(params)
        in_maps.append(m)

    kwargs = {}
    if TRACE:
        kwargs = dict(trace=True, stitch_traces=False)
    res = run_bass_kernel_spmd(nc, in_maps, core_ids=list(range(NCORES)),
                               **kwargs)
    global LAST_RESULTS
    LAST_RESULTS = res
    y = np.empty_like(u)
    for c in range(NCORES):
        yt = res.results[c]["y"].reshape(Q, BPC, 128, 8, 512)
        y[c * BPC:(c + 1) * BPC] = yt.transpose(1, 0, 3, 2, 4).reshape(
            BPC, L, u.shape[2])
    return y


# revision 41
# speedup vs baseline: 1.1030x; 1.0333x over previous
"""DPLR-SSM layer kernel for Trainium2 (8 NeuronCores, batch-parallel).

Math: the reference recurrence is
    x_t = M x_{t-1} + B_bar u_t,   M = diag(A_bar) + dt * P Q^H   (n=64 complex)
    y_t = Re(C x_t) + D * u_t
M is time-invariant, so we eigendecompose M = V diag(w) V^{-1} on the host
(tiny, n=64) and run the diagonal system
    x'_t = w x'_{t-1} + B_eff u_t,  y_t = Re(C_eff x'_t) + D u_t
with B_eff = V^{-1} B_bar, C_eff = C V.  The complex diagonal scan is made
real by the phase-rotation trick: with w = rho * e^{i*theta},
z_t = e^{-i*theta*t} x'_t obeys  z_t = rho * z_{t-1} + e^{-i*theta*t} b_t,
which is two independent REAL first-order scans (hardware tensor_tensor_scan,
fp32 carry state, bf16 storage).

Per-core layout (2 batches of the 16), all heavy tensors bf16:
  - u is uploaded twice, already in bf16: pre-transposed [d, t] (PE matmul
    operand for the B-projection, no on-device transposes) and as the
    feedthrough term du = D*u in natural [t, d] layout (D is a constant
    diagonal, folded into input preparation on the host).
  - B-projection: binb[(comp,n), t] = sum_d Bc[d,(comp,n)] * uT[d, t]  (PE,
    4 accumulating chunk matmuls per 512-t group); binb2 is the
    component-swapped projection (second matmul set).
  - pre-rotation (plain 2D per-batch ops so the DVE 2x bf16 mode engages):
        bp = [c;c] * binb - [-s;s] * binb2 = [c*br + s*bi ; c*bi - s*br]
    tmp2 ( [-s;s]*binb2 ) and the post-rotation g2 run on GpSimd, which is
    otherwise idle, to shorten the DVE critical path.
  - hardware scans (state chained across passes via zprev column).
  - post-rotation G1 = [c;s]*z, G2 = [-s;c]*z; C-projection
    y[t,d] = G1.T W1 + G2.T W2 accumulated in PSUM.
  - merge: ACT evacuates the C-proj PSUM to bf16, DVE adds du in an
    all-bf16 2x op, and the store DMA casts bf16->fp32 (software DGE on
    gpsimd; y is written in a pre-tiled layout the host de-tiles).
Processed as Q=4 time-quarter passes with pass q+1's loads issued ahead of
pass q's stores (loads on the sync queue, stores on the gpsimd queue).
"""

import math

import numpy as np

N = 64
D = 512
BATCH = 16
SEQ = 4096
NCORES = 8
BPC = BATCH // NCORES  # batches per core = 2

_PROG_CACHE = {}

# Set by test harnesses to capture a hardware profile; harmless defaults.
TRACE = False
LAST_RESULTS = None


def _host_precompute(log_neg_real, imag, P_real, P_imag, Q_real, Q_imag,
                     B_real, B_imag, C_real, C_imag, log_dt, D_vec, L):
    """All small-parameter math in float64 on host; returns device arrays."""
    import ml_dtypes
    bf16 = ml_dtypes.bfloat16

    dt = math.exp(float(np.asarray(log_dt).reshape(-1)[0]))
    Lam = -np.exp(log_neg_real.astype(np.float64)) + 1j * imag.astype(np.float64)
    A_bar = np.exp(Lam * dt)
    B = B_real.astype(np.float64) + 1j * B_imag.astype(np.float64)
    B_bar = ((A_bar - 1.0) / (Lam + 1e-8) * dt)[:, None] * B          # (n, d)
    P = P_real.astype(np.float64) + 1j * P_imag.astype(np.float64)
    Qc = Q_real.astype(np.float64) - 1j * Q_imag.astype(np.float64)
    C = C_real.astype(np.float64) + 1j * C_imag.astype(np.float64)   # (d, n)

    M = np.diag(A_bar) + dt * (P @ Qc.T)
    w, V = np.linalg.eig(M)
    B_eff = np.linalg.solve(V, B_bar)                                 # (n, d)
    C_eff = C @ V                                                     # (d, n)

    rho = np.abs(w)
    theta = np.angle(w)
    t_idx = np.arange(1, L + 1, dtype=np.float64)
    ang = np.outer(theta, t_idx)                                      # (n, L)
    cos_t = np.cos(ang)
    sin_t = np.sin(ang)

    # post-rotation tables (128, L): t1 = [cos; sin], t2 = [-sin; cos]
    t1 = np.concatenate([cos_t, sin_t], axis=0).astype(bf16)
    t2 = np.concatenate([-sin_t, cos_t], axis=0).astype(bf16)
    # pre-rotation table for tmp2: tsn = [-sin; sin]  (tmp1's [cos; cos]
    # comes from half-width ops on t1/t2 instead of a fourth table)
    tsn = np.concatenate([-sin_t, sin_t], axis=0).astype(bf16)

    # rho column (128, 1) fp32: per-partition scan coefficient
    rhoc = np.concatenate([rho, rho]).astype(np.float32).reshape(128, 1)

    # B weights, lhsT layout: bcomb[p, c*128+m] = Bc[c*128+p, m]
    # where Bc[d, m] with m=comp*64+n: comp0 -> Re(B_eff)[n,d], comp1 -> Im
    Bc = np.concatenate([B_eff.real, B_eff.imag], axis=0).T           # (512, 128)
    bcomb = Bc.reshape(4, 128, 128).transpose(1, 0, 2).reshape(128, 512)
    bcomb = np.ascontiguousarray(bcomb).astype(bf16)
    # component-swapped variant [bi ; br]
    Bc2 = np.concatenate([B_eff.imag, B_eff.real], axis=0).T          # (512, 128)
    bcomb2 = Bc2.reshape(4, 128, 128).transpose(1, 0, 2).reshape(128, 512)
    bcomb2 = np.ascontiguousarray(bcomb2).astype(bf16)

    # C-proj weights: W1 = [Cr; -Cr], W2 = [Ci; -Ci]
    Cr = C_eff.real.T                                                 # (n, d)
    Ci = C_eff.imag.T
    W1 = np.concatenate([Cr, -Cr], axis=0)                            # (128, 512)
    W2 = np.concatenate([Ci, -Ci], axis=0)
    cexp = np.concatenate([W1, W2], axis=1).astype(bf16)              # (128, 1024)

    return dict(t1=t1, t2=t2, tsn=tsn, rhoc=rhoc,
                bcomb=bcomb, bcomb2=bcomb2, cexp=cexp)


def _split_multi_waits(nc, mybir):
    """Walrus codegen only honors a single sync-wait slot on compute
    instruction structs (ACT/TS/TT...).  Move surplus waits onto chained
    EventSemaphore instructions on the same engine right before the op --
    in-order engine execution makes this equivalent."""
    n = 0
    for func in nc.m.functions:
        for blk in func.blocks:
            il = blk.instructions
            i = 0
            while i < len(il):
                inst = il[i]
                si = inst.sync_info
                if (si is not None and si.on_wait and len(si.on_wait) > 1
                        and not isinstance(inst, mybir.InstEventSemaphore)):
                    waits = list(si.on_wait)
                    for w in waits[:-1]:
                        ev = mybir.InstEventSemaphore(
                            name=f"EVW-{n}", ins=[], outs=[])
                        n += 1
                        ev.engine = inst.engine
                        ev.sync_info = mybir.SyncInfo(on_wait=[w],
                                                      on_update=[])
                        il.insert(i, ev)
                        i += 1
                    inst.sync_info = mybir.SyncInfo(on_wait=[waits[-1]],
                                                    on_update=si.on_update)
                i += 1
    return n


def _build_program(L, split_waits=True):
    """SPMD Bass program for one core: u (BPC*L, 512) -> y (BPC*L, 512) fp32.
    Q=4 time-quarter passes, loads for pass q+1 prefetched before pass q's
    compute is emitted."""
    import concourse.bass as bass
    import concourse.mybir as mybir
    import concourse.tile as tile

    TROWS = BPC * L            # 8192 time-rows per core
    Q = 4                      # passes (time quarters)
    TQ = L // Q                # 1024 time steps per pass per batch
    FP32 = mybir.dt.float32
    BF16 = mybir.dt.bfloat16
    Alu = mybir.AluOpType

    nc = bass.Bass()
    # du/ut/y are tiled on the host so each per-pass DMA is one plain 2D
    # transfer with a KB-contiguous row per partition (128 descriptors):
    #   du row (q*BPC+b)*128+p holds du[b, q*TQ + j*128 + p, :] for j=0..7
    #   ut row (q*BPC+b)*128+p holds u[b, q*TQ + t, c*128+p] g-major:
    #     col g*2048 + c*512 + t' = u[.., q*TQ + g*512 + t', c*128+p]
    #   y  row (q*BPC+b)*128+p holds y[b, q*TQ + j*128 + p, :] for j=0..7
    du_d = nc.dram_tensor("du", [Q * BPC * 128, 8 * 512], BF16,
                          kind="ExternalInput")
    ut_d = nc.dram_tensor("ut", [Q * BPC * 128, 4 * (L // Q)], BF16,
                          kind="ExternalInput")
    t1_d = nc.dram_tensor("t1", [128, L], BF16, kind="ExternalInput")
    t2_d = nc.dram_tensor("t2", [128, L], BF16, kind="ExternalInput")
    tsn_d = nc.dram_tensor("tsn", [128, L], BF16, kind="ExternalInput")
    rhoc_d = nc.dram_tensor("rhoc", [128, 1], FP32, kind="ExternalInput")
    bcomb_d = nc.dram_tensor("bcomb", [128, 512], BF16, kind="ExternalInput")
    bcomb2_d = nc.dram_tensor("bcomb2", [128, 512], BF16, kind="ExternalInput")
    cexp_d = nc.dram_tensor("cexp", [128, 1024], BF16, kind="ExternalInput")
    y_d = nc.dram_tensor("y", [Q * BPC * 128, 8 * 512], FP32,
                         kind="ExternalOutput")

    with tile.TileContext(nc) as tc_:
        with (
            tc_.tile_pool(name="persist", bufs=1) as pp,
            tc_.tile_pool(name="ptab", bufs=2) as ptab,
            tc_.tile_pool(name="put", bufs=4) as put,
            tc_.tile_pool(name="pun", bufs=4) as pun,
            tc_.tile_pool(name="pbin", bufs=2) as pbin,
            tc_.tile_pool(name="ptmp", bufs=2) as ptmp,
            tc_.tile_pool(name="pz", bufs=2) as pz,
            tc_.tile_pool(name="pg", bufs=2) as pg,
            tc_.tile_pool(name="pyo", bufs=7) as pyo,
            tc_.tile_pool(name="pys", bufs=4) as pys,
            tc_.tile_pool(name="psB", bufs=1, space="PSUM") as psB,
            tc_.tile_pool(name="psC", bufs=3, space="PSUM") as psC,
        ):
            bcomb_s = pp.tile([128, 512], BF16, tag="bcomb")
            bcomb2_s = pp.tile([128, 512], BF16, tag="bcomb2")
            cexp_s = pp.tile([128, 1024], BF16, tag="cexp")
            rhoc_s = pp.tile([128, 1], FP32, tag="rhoc")
            zprev = pp.tile([128, BPC], FP32, tag="zprev")
            # small persistent params go on the scalar queue so the sync
            # queue starts streaming pass-0 u immediately
            nc.scalar.dma_start(out=bcomb_s, in_=bcomb_d[:, :])
            nc.scalar.dma_start(out=bcomb2_s, in_=bcomb2_d[:, :])
            nc.scalar.dma_start(out=cexp_s, in_=cexp_d[:, :])
            nc.scalar.dma_start(out=rhoc_s, in_=rhoc_d[:, :])
            nc.gpsimd.memset(zprev, 0.0)

            rhob = rhoc_s.broadcast_to([128, TQ])

            def emit_loads(q):
                """u loads (sync queue) + table loads (scalar queue)."""
                cs = slice(q * TQ, (q + 1) * TQ)
                tiles = {}
                for b in range(BPC):
                    uT = put.tile([128, 4 * TQ], BF16, tag="uT")
                    rows = slice((q * BPC + b) * 128, (q * BPC + b + 1) * 128)
                    # two halves so the first B-proj group can start after
                    # half the tile has landed (g-major layout)
                    half = 2 * TQ
                    nc.sync.dma_start(out=uT[:, 0:half],
                                      in_=ut_d[rows, 0:half])
                    nc.sync.dma_start(out=uT[:, half:2 * half],
                                      in_=ut_d[rows, half:2 * half])
                    tiles[("uT", b)] = uT
                # tables before du: the rotation needs them early, while du
                # is only consumed by the output stage at the end of a pass
                for name, dram in (("t1", t1_d), ("t2", t2_d),
                                   ("tsn", tsn_d)):
                    ts_ = ptab.tile([128, TQ], BF16, tag=name)
                    nc.sync.dma_start(out=ts_, in_=dram[:, cs])
                    tiles[name] = ts_
                for b in range(BPC):
                    du = pun.tile([128, 8 * 512], BF16, tag="du")
                    rows = slice((q * BPC + b) * 128, (q * BPC + b + 1) * 128)
                    nc.sync.dma_start(out=du, in_=du_d[rows, :])
                    tiles[("du", b)] = du
                return tiles

            tiles = emit_loads(0)
            for q in range(Q):
                nxt = emit_loads(q + 1) if q + 1 < Q else None
                cur = tiles

                # ---------------- B-projection (PE) ------------------------
                binb_s = pbin.tile([128, BPC * TQ], BF16, tag="binb")
                binb2_s = pbin.tile([128, BPC * TQ], BF16, tag="binb2")
                for b in range(BPC):
                    uT = cur[("uT", b)]
                    for g in range(TQ // 512):
                        pb = psB.tile([128, 512], FP32, tag="pb")
                        pb2 = psB.tile([128, 512], FP32, tag="pb2")
                        for c in range(4):
                            rhs = uT[:, g * 2048 + c * 512:
                                     g * 2048 + (c + 1) * 512]
                            nc.tensor.matmul(
                                pb, bcomb_s[:, c * 128:(c + 1) * 128], rhs,
                                start=(c == 0), stop=(c == 3))
                        for c in range(4):
                            rhs = uT[:, g * 2048 + c * 512:
                                     g * 2048 + (c + 1) * 512]
                            nc.tensor.matmul(
                                pb2, bcomb2_s[:, c * 128:(c + 1) * 128], rhs,
                                start=(c == 0), stop=(c == 3))
                        off = b * TQ + g * 512
                        nc.scalar.copy(binb_s[:, off:off + 512], pb)
                        nc.scalar.copy(binb2_s[:, off:off + 512], pb2)

                # ---------------- rotation + scan (DVE + GpSimd) -----------
                # plain 2D per-batch slices keep the DVE 2x bf16 mode; the
                # tmp2/g2 products run on the otherwise-idle GpSimd
                tmp1 = ptmp.tile([128, BPC * TQ], BF16, tag="tmp1")
                tmp2 = ptmp.tile([128, BPC * TQ], BF16, tag="tmp2")
                sls = [slice(b * TQ, (b + 1) * TQ) for b in range(BPC)]
                for b in range(BPC):
                    # tmp1 = [cos;cos]*binb via aligned half-lane products:
                    # t1 rows 0:64 and t2 rows 64:128 are both cos
                    nc.vector.tensor_mul(tmp1[0:64, sls[b]],
                                         cur["t1"][0:64, :],
                                         binb_s[0:64, sls[b]])
                    nc.vector.tensor_mul(tmp1[64:128, sls[b]],
                                         cur["t2"][64:128, :],
                                         binb_s[64:128, sls[b]])
                    nc.gpsimd.tensor_mul(tmp2[:, sls[b]], cur["tsn"][:, :],
                                         binb2_s[:, sls[b]])
                # bp overwrites binb_s (its inputs are tmp1/tmp2); per-batch
                # so scan(b0) is not gated on batch 1's products
                for b in range(BPC):
                    nc.vector.tensor_sub(binb_s[:, sls[b]],
                                         tmp1[:, sls[b]], tmp2[:, sls[b]])

                z = pz.tile([128, BPC * TQ], BF16, tag="z")
                for b in range(BPC):
                    nc.vector.tensor_tensor_scan(
                        z[:, sls[b]], rhob, binb_s[:, sls[b]],
                        zprev[:, b:b + 1], Alu.mult, Alu.add)
                    nc.vector.tensor_copy(
                        zprev[:, b:b + 1],
                        z[:, b * TQ + TQ - 1:b * TQ + TQ])

                g1 = pg.tile([128, BPC * TQ], BF16, tag="g1")
                g2 = pg.tile([128, BPC * TQ], BF16, tag="g2")
                for b in range(BPC):
                    nc.vector.tensor_mul(g1[:, sls[b]], cur["t1"][:, :],
                                         z[:, sls[b]])
                    nc.gpsimd.tensor_mul(g2[:, sls[b]], cur["t2"][:, :],
                                         z[:, sls[b]])

                # ---------------- C-projection + feedthrough + store -------
                # PSUM is evacuated to bf16 (ACT), the du add runs as an
                # all-bf16 DVE 2x op, and the store casts bf16->fp32 in the
                # DMA (software DGE on gpsimd; y is de-tiled on the host)
                for b in range(BPC):
                    du = cur[("du", b)]
                    yrows = slice((q * BPC + b) * 128,
                                  (q * BPC + b + 1) * 128)
                    for h in range(2):        # 512-row output halves
                        yout = pyo.tile([128, 2048], BF16, tag="yout")
                        for jp in range(2):   # pairs of 128-t psum tiles
                            py = psC.tile([128, 1024], FP32, tag="py")
                            for ji in range(2):
                                j = h * 4 + jp * 2 + ji
                                off = b * TQ + j * 128
                                ps_ = py[:, ji * 512:(ji + 1) * 512]
                                nc.tensor.matmul(ps_, g1[:, off:off + 128],
                                                 cexp_s[:, 0:512],
                                                 start=True, stop=False)
                                nc.tensor.matmul(ps_, g2[:, off:off + 128],
                                                 cexp_s[:, 512:1024],
                                                 start=False, stop=True)
                            # per-jp evac + add: the add for jp0 overlaps
                            # the ACT evac for jp1 (finer pipelining beats
                            # the saved op overhead of a merged add)
                            ysum = pys.tile([128, 1024], BF16, tag="ysum")
                            nc.scalar.copy(ysum, py)
                            js = slice(jp * 1024, (jp + 1) * 1024)
                            nc.vector.tensor_add(
                                yout[:, js], ysum,
                                du[:, h * 2048 + jp * 1024:
                                   h * 2048 + (jp + 1) * 1024])
                            # store each half right after its add lands
                            nc.gpsimd.dma_start(
                                out=y_d[yrows,
                                        h * 2048 + jp * 1024:
                                        h * 2048 + (jp + 1) * 1024],
                                in_=yout[:, js])

                tiles = nxt

    if split_waits:
        _split_multi_waits(nc, mybir)
    return nc


def kernel(**inputs):
    import ml_dtypes
    from concourse.bass_utils import run_bass_kernel_spmd

    bf16 = ml_dtypes.bfloat16
    u = np.ascontiguousarray(inputs["u"], dtype=np.float32)
    L = u.shape[1]
    params = _host_precompute(
        inputs["log_neg_real"], inputs["imag"], inputs["P_real"],
        inputs["P_imag"], inputs["Q_real"], inputs["Q_imag"],
        inputs["B_real"], inputs["B_imag"], inputs["C_real"],
        inputs["C_imag"], inputs["log_dt"], inputs["D"], L)

    if L not in _PROG_CACHE:
        _PROG_CACHE[L] = _build_program(L)
    nc = _PROG_CACHE[L]

    u16 = u.astype(bf16)
    du16 = (u * inputs["D"].astype(np.float32)[None, None, :]).astype(bf16)
    Q = 4
    TQ = L // Q
    in_maps = []
    for c in range(NCORES):
        shard = u16[c * BPC:(c + 1) * BPC].reshape(BPC * L, u.shape[2])
        dshard = du16[c * BPC:(c + 1) * BPC].reshape(BPC * L, u.shape[2])
        # pre-tiled layouts (see _build_program): one 8KB-contiguous row per
        # (pass, batch, partition)
        dut = dshard.reshape(BPC, Q, 8, 128, 512).transpose(
            1, 0, 3, 2, 4).reshape(Q * BPC * 128, 8 * 512)
        utt = np.ascontiguousarray(shard.T).reshape(
            4, 128, BPC, Q, 2, 512).transpose(
            3, 2, 1, 4, 0, 5).reshape(Q * BPC * 128, 4 * TQ)
        m = {"du": np.ascontiguousarray(dut),
             "ut": np.ascontiguousarray(utt)}
        m.update(params)
        in_maps.append(m)

    kwargs = {}
    if TRACE:
        kwargs = dict(trace=True, stitch_traces=False)
    res = run_bass_kernel_spmd(nc, in_maps, core_ids=list(range(NCORES)),
                               **kwargs)
    global LAST_RESULTS
    LAST_RESULTS = res
    y = np.empty_like(u)
    for c in range(NCORES):
        yt = res.results[c]["y"].reshape(Q, BPC, 128, 8, 512)
        y[c * BPC:(c + 1) * BPC] = yt.transpose(1, 0, 3, 2, 4).reshape(
            BPC, L, u.shape[2])
    return y
